# revision 13
# baseline (speedup 1.0000x reference)
"""Trainium2 Bass kernel for nn_DeepFeatureLoss (pairwise softmax-correspondence loss).

Math (per batch b):
    P = softmax_j(-||x_i - x_j||^2),   x = points / SIGMA
    F = softmax_j(-||f1_i - f2_j||^2)
    out[b] = sum_i w_i * sum_j (P_ij - F_ij)^2

Strategy: shard rows i across 8 cores (512 rows each). Host precomputes
transposed/augmented matmul operands so the device kernel is pure compute:
    score_spatial[i,j] = (2/s^2) x_i.x_j - (1/s^2)|x_j|^2   (K=4 matmul, ones row)
    exp with per-row bias -(1/s^2)|x_i|^2  ->  exp(-||xi-xj||^2/s^2)  (<= 1, no overflow)
and similarly for features (K=33). Row sums come free via activation accum.
    sum_j (P-F)^2 = (1/s1^2) * sum_j (c*e2 - e1)^2,  c = s1/s2
computed with one fused scalar_tensor_tensor pass + split square-reduce
(ScalarE on the first 1024 cols, VectorE tensor_tensor_reduce on the rest).
Per-core partial losses [128 lanes, B] are summed on host.
"""

import os
import sys

import numpy as np

sys.path.insert(0, "/opt/trn_rl_repo")

import concourse.bass as bass
import concourse.tile as tile
from concourse import mybir
from concourse.bass_utils import run_bass_kernel_spmd

SIGMA = 0.05
B = 2
N = 4096
D = 32
NCORES = 8
RPC = N // NCORES          # rows per core = 512
TILES = RPC // 128         # i-tiles per core per batch = 4
KF = D + 1                 # feature contraction with ones row
KS = 4                     # spatial contraction (3 coords + ones row)
ACT_COLS = 1024            # columns of the square-reduce done on ScalarE

FP = mybir.dt.float32
FPR = mybir.dt.float32r    # 4x faster PE streaming, fp32 data
AX = mybir.AxisListType
OP = mybir.AluOpType
AF = mybir.ActivationFunctionType

LAST_RESULT = None         # test harness introspection


def _fix_walrus_incompat(nc):
    """This container's walrus codegen fits exactly ONE sync-wait per engine
    instruction struct (Tile's scheduler freely emits several) and rejects the
    EVENT_SEMAPHORE_RANGE_CLEAR raw-ISA instruction Tile emits at context
    exit. Rewrite: (a) every multi-wait instruction becomes (n-1) same-engine
    EventSemaphore waits followed by the instruction with the final wait;
    (b) the range-clear becomes one sem-wr-imm(0) EventSemaphore per sem."""
    import re

    from bass_rust import SyncInfo, SyncUpdate

    fn = nc.m.functions[0]
    originals = [(blk, list(blk.instructions)) for blk in fn.blocks]
    rebuilt = []
    for blk, insts in originals:
        out = []
        for inst in insts:
            tname = type(inst).__name__
            si = inst.sync_info
            if tname == "InstISA" and "EVENT_SEMAPHORE_RANGE_CLEAR" in inst.concise():
                m = re.search(r"range_first=(\d+) range_last=(\d+)", inst.concise())
                first, last = int(m.group(1)), int(m.group(2))
                for sem in range(first, last + 1):
                    ev = mybir.InstEventSemaphore(
                        name=nc.get_next_instruction_name(),
                        engine=inst.engine,
                        sync_info=SyncInfo(
                            on_wait=list(si.on_wait) if si and sem == first else [],
                            on_update=[
                                SyncUpdate(
                                    sync_type="semaphore",
                                    id=sem,
                                    ant_name=f"semclear_{sem}",
                                    update_mode="sem-wr-imm",
                                    update_value=0,
                                    update_reg=None,
                                )
                            ],
                        ),
                    )
                    nc.register_instruction(ev, overwrite=True)
                    out.append(ev)
                continue
            if si is not None and len(si.on_wait) > 1:
                waits = list(si.on_wait)
                for w in waits[:-1]:
                    ev = mybir.InstEventSemaphore(
                        name=nc.get_next_instruction_name(),
                        engine=inst.engine,
                        sync_info=SyncInfo(on_wait=[w], on_update=[]),
                    )
                    nc.register_instruction(ev, overwrite=True)
                    out.append(ev)
                inst.sync_info = SyncInfo(
                    on_wait=[waits[-1]], on_update=list(si.on_update)
                )
            out.append(inst)
        rebuilt.append((blk, out))
    for blk, out in rebuilt:
        blk.instructions[:] = out


def _build_nc(mm_dtype):
    nc = bass.Bass()

    # rhs (cols 0:N) and the local lhsT block (cols N:N+RPC) share one DRAM
    # tensor so each matmul family depends on exactly ONE input DMA — walrus's
    # core_v3 LDWEIGHTS struct only fits a single sync-wait.
    spat_comb = nc.dram_tensor("spat_comb", [B, KS, N + RPC], mm_dtype, kind="ExternalInput")
    feat_comb = nc.dram_tensor("feat_comb", [B, KF, N + RPC], mm_dtype, kind="ExternalInput")
    bias_x = nc.dram_tensor("bias_x", [B, RPC], FP, kind="ExternalInput")
    bias_f = nc.dram_tensor("bias_f", [B, RPC], FP, kind="ExternalInput")
    wloc = nc.dram_tensor("wloc", [B, RPC], FP, kind="ExternalInput")
    out = nc.dram_tensor("out", [B, 128], FP, kind="ExternalOutput")

    with tile.TileContext(nc) as tc:
        with (
            tc.tile_pool(name="const", bufs=1) as cpool,
            tc.tile_pool(name="psum", bufs=2, space="PSUM") as ppool,
            tc.tile_pool(name="ebuf", bufs=2) as epool,
            tc.tile_pool(name="junk", bufs=2) as jpool,
            tc.tile_pool(name="small", bufs=3) as spool,
            tc.tile_pool(name="accs", bufs=1) as apool,
        ):
            # --- load per-batch constants ---
            scomb, fcomb, bx, bf, wt = [], [], [], [], []
            for b in range(B):
                t_ = cpool.tile([KS, N + RPC], mm_dtype, tag=f"scomb{b}")
                nc.sync.dma_start(t_[:], spat_comb[b])
                scomb.append(t_)
                t_ = cpool.tile([KF, N + RPC], mm_dtype, tag=f"fcomb{b}")
                nc.sync.dma_start(t_[:], feat_comb[b])
                fcomb.append(t_)
                for lst, src, tag in ((bx, bias_x, "bx"), (bf, bias_f, "bf"), (wt, wloc, "wt")):
                    t_ = cpool.tile([128, TILES], FP, tag=f"{tag}{b}")
                    nc.sync.dma_start(t_[:], src[b].rearrange("(t p) -> p t", p=128))
                    lst.append(t_)

            outsb = apool.tile([128, B], FP, tag="outsb")

            for b in range(B):
                accq = apool.tile([128, TILES], FP, tag=f"accq{b}")
                for t in range(TILES):
                    spart = spool.tile([128, 4], FP, tag="spart")
                    e1 = epool.tile([128, N], FP, tag="e1")
                    e2 = epool.tile([128, N], FP, tag="e2")
                    # spatial & feature score blocks + exp, 2048 cols at a time
                    for half, (comb_t, bias_t, ebuf) in enumerate(
                        (
                            (scomb[b], bx[b], e1),
                            (scomb[b], bx[b], e1),
                            (fcomb[b], bf[b], e2),
                            (fcomb[b], bf[b], e2),
                        )
                    ):
                        half_is = half % 2
                        col0 = half_is * 2048
                        ps = ppool.tile([128, 2048], FP, tag="ps")
                        for k in range(4):
                            nc.tensor.matmul(
                                ps[:, k * 512 : (k + 1) * 512],
                                comb_t[:, N + t * 128 : N + (t + 1) * 128],
                                comb_t[:, col0 + k * 512 : col0 + (k + 1) * 512],
                                start=True,
                                stop=True,
                            )
                        nc.scalar.activation(
                            ebuf[:, col0 : col0 + 2048],
                            ps[:],
                            AF.Exp,
                            bias=bias_t[:, t : t + 1],
                            accum_out=spart[:, 2 * (half // 2) + half_is : 2 * (half // 2) + half_is + 1],
                        )
                    # row sums s1 (spatial), s2 (feature) and derived scalars
                    sums = spool.tile([128, 2], FP, tag="sums")
                    nc.vector.tensor_reduce(
                        sums[:], spart[:].rearrange("p (m c) -> p m c", c=2), axis=AX.X, op=OP.add
                    )
                    rec = spool.tile([128, 2], FP, tag="rec")
                    nc.vector.reciprocal(rec[:], sums[:])
                    cc = spool.tile([128, 1], FP, tag="cc")
                    nc.vector.tensor_tensor(cc[:], sums[:, 0:1], rec[:, 1:2], op=OP.mult)
                    # d = c*e2 - e1 (in place into e2); then sum d^2 split S/V
                    nc.vector.scalar_tensor_tensor(
                        e2[:], e2[:], cc[:], e1[:], op0=OP.mult, op1=OP.subtract
                    )
                    qa = spool.tile([128, 1], FP, tag="qa")
                    qv = spool.tile([128, 1], FP, tag="qv")
                    junk = jpool.tile([128, ACT_COLS], FP, tag="junk")
                    nc.scalar.activation(
                        junk[:],
                        e2[:, 0:ACT_COLS],
                        AF.Square,
                        scale=rec[:, 0:1],
                        accum_out=qa[:],
                    )
                    nc.vector.scalar_tensor_tensor(
                        out=e2[:, ACT_COLS:],
                        in0=e2[:, ACT_COLS:],
                        scalar=1.0,
                        in1=e2[:, ACT_COLS:],
                        op0=OP.mult,
                        op1=OP.mult,
                        accum_out=qv[:],
                    )
                    # loss rows = qa + inv1^2 * qv
                    i2 = spool.tile([128, 1], FP, tag="i2")
                    nc.vector.tensor_tensor(i2[:], rec[:, 0:1], rec[:, 0:1], op=OP.mult)
                    qv2 = spool.tile([128, 1], FP, tag="qv2")
                    nc.vector.tensor_tensor(qv2[:], qv[:], i2[:], op=OP.mult)
                    nc.vector.tensor_tensor(accq[:, t : t + 1], qa[:], qv2[:], op=OP.add)
                # weighted reduce over this batch's 4 i-tiles
                lw = spool.tile([128, TILES], FP, tag="lw")
                nc.vector.tensor_tensor(lw[:], accq[:], wt[b][:], op=OP.mult)
                nc.vector.tensor_reduce(outsb[:, b : b + 1], lw[:], axis=AX.X, op=OP.add)

            for b in range(B):
                nc.sync.dma_start(out[b].rearrange("(p o) -> p o", o=1), outsb[:, b : b + 1])

    _fix_walrus_incompat(nc)
    return nc


_NC_CACHE = {}


def _get_nc(use_fp32r=True):
    key = "r" if use_fp32r else "f"
    if key not in _NC_CACHE:
        _NC_CACHE[key] = _build_nc(FPR if use_fp32r else FP)
    return _NC_CACHE[key]


def _prep_inputs(points, pointfea1, pointfea2, weights):
    """Host-side sharding + operand layout. Returns per-core input maps."""
    s2inv = np.float64(1.0) / (SIGMA * SIGMA)
    x = points.astype(np.float64)        # [B, N, 3]
    f1 = pointfea1.astype(np.float64)    # [B, N, D]
    f2 = pointfea2.astype(np.float64)
    w = weights.astype(np.float32)

    xT = np.swapaxes(x, 1, 2)            # [B, 3, N]
    f1T = np.swapaxes(f1, 1, 2)          # [B, D, N]
    f2T = np.swapaxes(f2, 1, 2)

    xn = np.sum(x * x, axis=2)           # [B, N]
    f1n = np.sum(f1 * f1, axis=2)
    f2n = np.sum(f2 * f2, axis=2)

    in_maps = []
    for c in range(NCORES):
        sl = slice(c * RPC, (c + 1) * RPC)
        spat_comb = np.empty((B, KS, N + RPC), np.float32)
        spat_comb[:, :3, :N] = 2.0 * s2inv * xT
        spat_comb[:, 3, :N] = -s2inv * xn
        spat_comb[:, :3, N:] = xT[:, :, sl]
        spat_comb[:, 3, N:] = 1.0
        feat_comb = np.empty((B, KF, N + RPC), np.float32)
        feat_comb[:, :D, :N] = 2.0 * f2T
        feat_comb[:, D, :N] = -f2n
        feat_comb[:, :D, N:] = f1T[:, :, sl]
        feat_comb[:, D, N:] = 1.0
        in_maps.append(
            {
                "spat_comb": spat_comb,
                "feat_comb": feat_comb,
                "bias_x": (-s2inv * xn[:, sl]).astype(np.float32),
                "bias_f": (-f1n[:, sl]).astype(np.float32),
                "wloc": np.ascontiguousarray(w[:, sl]),
            }
        )
    return in_maps


def kernel(points, pointfea1, pointfea2, weights):
    global LAST_RESULT
    in_maps = _prep_inputs(points, pointfea1, pointfea2, weights)
    nc = _get_nc(use_fp32r=os.environ.get("DFL_NO_FP32R", "") != "1")
    res = run_bass_kernel_spmd(nc, in_maps, core_ids=list(range(NCORES)))
    LAST_RESULT = res
    total = np.zeros(B, np.float64)
    for m in res.results:
        total += m["out"].astype(np.float64).sum(axis=1)
    return total.astype(np.float32)


# revision 18
# speedup vs baseline: 2.2676x; 2.2676x over previous
"""Trainium2 Bass kernel for nn_DeepFeatureLoss (pairwise softmax-correspondence loss).

Math (per batch b):
    P = softmax_j(-||x_i - x_j||^2),   x = points / SIGMA
    F = softmax_j(-||f1_i - f2_j||^2)
    out[b] = sum_i w_i * sum_j (P_ij - F_ij)^2

Strategy: shard rows i across 8 cores (512 rows each). Host precomputes
transposed/augmented matmul operands so the device kernel is pure compute:
    score_spatial[i,j] = (2/s^2) x_i.x_j - (1/s^2)|x_j|^2   (K=4 matmul, ones row)
    exp with per-row bias -(1/s^2)|x_i|^2  ->  exp(-||xi-xj||^2/s^2)  (<= 1, no overflow)
and similarly for features (K=33). Row sums come free via activation accum.
    sum_j (P-F)^2 = (1/s1^2) * sum_j (c*e2 - e1)^2,  c = s1/s2
computed with one fused scalar_tensor_tensor pass + split square-reduce
(ScalarE on the first 1024 cols, VectorE tensor_tensor_reduce on the rest).
Per-core partial losses [128 lanes, B] are summed on host.
"""

import os
import sys

import numpy as np

sys.path.insert(0, "/opt/trn_rl_repo")

import concourse.bass as bass
import concourse.tile as tile
from concourse import mybir
from concourse.bass_utils import run_bass_kernel_spmd

SIGMA = 0.05
B = 2
N = 4096
D = 32
NCORES = 8
RPC = N // NCORES          # rows per core = 512
TILES = RPC // 128         # i-tiles per core per batch = 4
KF = D + 1                 # feature contraction with ones row
KS = 4                     # spatial contraction (3 coords + ones row)
ACT_COLS = 1024            # columns of the square-reduce done on ScalarE

FP = mybir.dt.float32
FPR = mybir.dt.float32r    # 4x faster PE streaming, fp32 data
AX = mybir.AxisListType
OP = mybir.AluOpType
AF = mybir.ActivationFunctionType

LAST_RESULT = None         # test harness introspection


def _fix_walrus_incompat(nc):
    """This container's walrus codegen fits exactly ONE sync-wait per engine
    instruction struct (Tile's scheduler freely emits several) and rejects the
    EVENT_SEMAPHORE_RANGE_CLEAR raw-ISA instruction Tile emits at context
    exit. Rewrite: (a) every multi-wait instruction becomes (n-1) same-engine
    EventSemaphore waits followed by the instruction with the final wait;
    (b) the range-clear becomes one sem-wr-imm(0) EventSemaphore per sem."""
    import re

    from bass_rust import SyncInfo, SyncUpdate

    fn = nc.m.functions[0]
    originals = [(blk, list(blk.instructions)) for blk in fn.blocks]
    rebuilt = []
    for blk, insts in originals:
        out = []
        for inst in insts:
            tname = type(inst).__name__
            si = inst.sync_info
            if tname == "InstISA" and "EVENT_SEMAPHORE_RANGE_CLEAR" in inst.concise():
                m = re.search(r"range_first=(\d+) range_last=(\d+)", inst.concise())
                first, last = int(m.group(1)), int(m.group(2))
                for sem in range(first, last + 1):
                    ev = mybir.InstEventSemaphore(
                        name=nc.get_next_instruction_name(),
                        engine=inst.engine,
                        sync_info=SyncInfo(
                            on_wait=list(si.on_wait) if si and sem == first else [],
                            on_update=[
                                SyncUpdate(
                                    sync_type="semaphore",
                                    id=sem,
                                    ant_name=f"semclear_{sem}",
                                    update_mode="sem-wr-imm",
                                    update_value=0,
                                    update_reg=None,
                                )
                            ],
                        ),
                    )
                    nc.register_instruction(ev, overwrite=True)
                    out.append(ev)
                continue
            if si is not None and len(si.on_wait) > 1:
                waits = list(si.on_wait)
                for w in waits[:-1]:
                    ev = mybir.InstEventSemaphore(
                        name=nc.get_next_instruction_name(),
                        engine=inst.engine,
                        sync_info=SyncInfo(on_wait=[w], on_update=[]),
                    )
                    nc.register_instruction(ev, overwrite=True)
                    out.append(ev)
                inst.sync_info = SyncInfo(
                    on_wait=[waits[-1]], on_update=list(si.on_update)
                )
            out.append(inst)
        rebuilt.append((blk, out))
    for blk, out in rebuilt:
        blk.instructions[:] = out


def _build_nc(spat_dtype, feat_dtype):
    nc = bass.Bass()

    # rhs (cols 0:N) and the local lhsT block (cols N:N+RPC) share one DRAM
    # tensor so each matmul family depends on exactly ONE input DMA — walrus's
    # core_v3 LDWEIGHTS struct only fits a single sync-wait.
    spat_comb = nc.dram_tensor("spat_comb", [B, KS, N + RPC], spat_dtype, kind="ExternalInput")
    feat_comb = nc.dram_tensor("feat_comb", [B, KF, N + RPC], feat_dtype, kind="ExternalInput")
    # biases + weights packed partition-major: smalls[p, tensor*2*TILES + b*TILES + t]
    # = value for row t*128+p of batch b. One contiguous 96B-per-partition DMA
    # instead of three 4B-scatter DMAs (which cost ~1 packet per element).
    smalls = nc.dram_tensor("smalls", [128, 3 * B * TILES], FP, kind="ExternalInput")
    out = nc.dram_tensor("out", [B, 128], FP, kind="ExternalOutput")

    with tile.TileContext(nc) as tc:
        with (
            tc.tile_pool(name="const", bufs=1) as cpool,
            tc.tile_pool(name="psum", bufs=2, space="PSUM") as ppool,
            tc.tile_pool(name="ebuf", bufs=2) as epool,
            tc.tile_pool(name="junk", bufs=2) as jpool,
            tc.tile_pool(name="small", bufs=3) as spool,
            tc.tile_pool(name="accs", bufs=1) as apool,
        ):
            # --- load constants ---
            sm = cpool.tile([128, 3 * B * TILES], FP, tag="smalls")
            nc.sync.dma_start(sm[:], smalls[:])
            bx = [sm[:, b * TILES : (b + 1) * TILES] for b in range(B)]
            bf = [sm[:, (B + b) * TILES : (B + b + 1) * TILES] for b in range(B)]
            wt = [sm[:, (2 * B + b) * TILES : (2 * B + b + 1) * TILES] for b in range(B)]
            scomb, fcomb = [], []
            for b in range(B):
                t_ = cpool.tile([KS, N + RPC], spat_dtype, tag=f"scomb{b}")
                nc.sync.dma_start(t_[:], spat_comb[b])
                scomb.append(t_)
                t_ = cpool.tile([KF, N + RPC], feat_dtype, tag=f"fcomb{b}")
                nc.sync.dma_start(t_[:], feat_comb[b])
                fcomb.append(t_)

            outsb = apool.tile([128, B], FP, tag="outsb")

            for b in range(B):
                accq = apool.tile([128, TILES], FP, tag=f"accq{b}")
                for t in range(TILES):
                    spart = spool.tile([128, 4], FP, tag="spart")
                    e1 = epool.tile([128, N], FP, tag="e1")
                    e2 = epool.tile([128, N], FP, tag="e2")
                    # spatial & feature score blocks + exp, 2048 cols at a time
                    for half, (comb_t, bias_t, ebuf) in enumerate(
                        (
                            (scomb[b], bx[b], e1),
                            (scomb[b], bx[b], e1),
                            (fcomb[b], bf[b], e2),
                            (fcomb[b], bf[b], e2),
                        )
                    ):
                        half_is = half % 2
                        col0 = half_is * 2048
                        ps = ppool.tile([128, 2048], FP, tag="ps")
                        for k in range(4):
                            nc.tensor.matmul(
                                ps[:, k * 512 : (k + 1) * 512],
                                comb_t[:, N + t * 128 : N + (t + 1) * 128],
                                comb_t[:, col0 + k * 512 : col0 + (k + 1) * 512],
                                start=True,
                                stop=True,
                            )
                        nc.scalar.activation(
                            ebuf[:, col0 : col0 + 2048],
                            ps[:],
                            AF.Exp,
                            bias=bias_t[:, t : t + 1],
                            accum_out=spart[:, 2 * (half // 2) + half_is : 2 * (half // 2) + half_is + 1],
                        )
                    # row sums s1 (spatial), s2 (feature) and derived scalars
                    sums = spool.tile([128, 2], FP, tag="sums")
                    nc.vector.tensor_reduce(
                        sums[:], spart[:].rearrange("p (m c) -> p m c", c=2), axis=AX.X, op=OP.add
                    )
                    rec = spool.tile([128, 2], FP, tag="rec")
                    nc.vector.reciprocal(rec[:], sums[:])
                    cc = spool.tile([128, 1], FP, tag="cc")
                    nc.vector.tensor_tensor(cc[:], sums[:, 0:1], rec[:, 1:2], op=OP.mult)
                    # d = c*e2 - e1 (in place into e2); then sum d^2 split S/V
                    nc.vector.scalar_tensor_tensor(
                        e2[:], e2[:], cc[:], e1[:], op0=OP.mult, op1=OP.subtract
                    )
                    qa = spool.tile([128, 1], FP, tag="qa")
                    qv = spool.tile([128, 1], FP, tag="qv")
                    junk = jpool.tile([128, ACT_COLS], FP, tag="junk")
                    nc.scalar.activation(
                        junk[:],
                        e2[:, 0:ACT_COLS],
                        AF.Square,
                        scale=rec[:, 0:1],
                        accum_out=qa[:],
                    )
                    nc.vector.scalar_tensor_tensor(
                        out=e2[:, ACT_COLS:],
                        in0=e2[:, ACT_COLS:],
                        scalar=1.0,
                        in1=e2[:, ACT_COLS:],
                        op0=OP.mult,
                        op1=OP.mult,
                        accum_out=qv[:],
                    )
                    # loss rows = qa + inv1^2 * qv
                    i2 = spool.tile([128, 1], FP, tag="i2")
                    nc.vector.tensor_tensor(i2[:], rec[:, 0:1], rec[:, 0:1], op=OP.mult)
                    qv2 = spool.tile([128, 1], FP, tag="qv2")
                    nc.vector.tensor_tensor(qv2[:], qv[:], i2[:], op=OP.mult)
                    nc.vector.tensor_tensor(accq[:, t : t + 1], qa[:], qv2[:], op=OP.add)
                # weighted reduce over this batch's 4 i-tiles
                lw = spool.tile([128, TILES], FP, tag="lw")
                nc.vector.tensor_tensor(lw[:], accq[:], wt[b], op=OP.mult)
                nc.vector.tensor_reduce(outsb[:, b : b + 1], lw[:], axis=AX.X, op=OP.add)

            for b in range(B):
                nc.sync.dma_start(out[b].rearrange("(p o) -> p o", o=1), outsb[:, b : b + 1])

    _fix_walrus_incompat(nc)
    return nc


_NC_CACHE = {}


def _get_nc(use_fp32r=True):
    """Default: fp32 spatial scores (magnitudes ~1e3 make fp32r's relaxed
    precision visible in the loss), fp32r feature scores (magnitudes ~1e2,
    error negligible). DFL_NO_FP32R=1 forces fp32 everywhere;
    DFL_ALL_FP32R=1 forces fp32r everywhere."""
    if os.environ.get("DFL_ALL_FP32R", "") == "1":
        key, dts = "rr", (FPR, FPR)
    elif not use_fp32r:
        key, dts = "ff", (FP, FP)
    else:
        key, dts = "fr", (FP, FPR)
    if key not in _NC_CACHE:
        _NC_CACHE[key] = _build_nc(*dts)
    return _NC_CACHE[key]


def _prep_inputs(points, pointfea1, pointfea2, weights):
    """Host-side sharding + operand layout. Returns per-core input maps."""
    s2inv = np.float64(1.0) / (SIGMA * SIGMA)
    x = points.astype(np.float64)        # [B, N, 3]
    f1 = pointfea1.astype(np.float64)    # [B, N, D]
    f2 = pointfea2.astype(np.float64)
    w = weights.astype(np.float32)

    xT = np.swapaxes(x, 1, 2)            # [B, 3, N]
    f1T = np.swapaxes(f1, 1, 2)          # [B, D, N]
    f2T = np.swapaxes(f2, 1, 2)

    xn = np.sum(x * x, axis=2)           # [B, N]
    f1n = np.sum(f1 * f1, axis=2)
    f2n = np.sum(f2 * f2, axis=2)

    in_maps = []
    for c in range(NCORES):
        sl = slice(c * RPC, (c + 1) * RPC)
        spat_comb = np.empty((B, KS, N + RPC), np.float32)
        spat_comb[:, :3, :N] = 2.0 * s2inv * xT
        spat_comb[:, 3, :N] = -s2inv * xn
        spat_comb[:, :3, N:] = xT[:, :, sl]
        spat_comb[:, 3, N:] = 1.0
        feat_comb = np.empty((B, KF, N + RPC), np.float32)
        feat_comb[:, :D, :N] = 2.0 * f2T
        feat_comb[:, D, :N] = -f2n
        feat_comb[:, :D, N:] = f1T[:, :, sl]
        feat_comb[:, D, N:] = 1.0
        # smalls[p, tensor*B*TILES + b*TILES + t] = value for row t*128+p
        smalls = np.empty((128, 3 * B * TILES), np.float32)
        for b in range(B):
            bx = (-s2inv * xn[b, sl]).astype(np.float32).reshape(TILES, 128)
            bfv = (-f1n[b, sl]).astype(np.float32).reshape(TILES, 128)
            wv = w[b, sl].reshape(TILES, 128)
            smalls[:, b * TILES : (b + 1) * TILES] = bx.T
            smalls[:, (B + b) * TILES : (B + b + 1) * TILES] = bfv.T
            smalls[:, (2 * B + b) * TILES : (2 * B + b + 1) * TILES] = wv.T
        in_maps.append(
            {
                "spat_comb": spat_comb,
                "feat_comb": feat_comb,
                "smalls": smalls,
            }
        )
    return in_maps


def kernel(points, pointfea1, pointfea2, weights):
    global LAST_RESULT
    in_maps = _prep_inputs(points, pointfea1, pointfea2, weights)
    nc = _get_nc(use_fp32r=os.environ.get("DFL_NO_FP32R", "") != "1")
    res = run_bass_kernel_spmd(nc, in_maps, core_ids=list(range(NCORES)))
    LAST_RESULT = res
    total = np.zeros(B, np.float64)
    for m in res.results:
        total += m["out"].astype(np.float64).sum(axis=1)
    return total.astype(np.float32)
